# revision 32
# baseline (speedup 1.0000x reference)
"""Discounted cumsum (B,H,S,D)=(8,16,4096,128), gamma per head, scan along S.

Strategy: batch-parallel across 8 NeuronCores (1 batch each, all 16 heads).
HBM traffic is the roofline, so I/O is bf16 (rel-err budget 2e-2; bf16 I/O
lands ~3e-3) and the host pre/post-transposes so every device DMA moves
fully contiguous 8KB partition lines.

Per head, X is laid out transposed [D=128 partitions, S=4096 free] and the
whole recurrence y[:, t] = g*y[:, t-1] + x[:, t] runs as ONE DVE
tensor_tensor_scan instruction (state fp32, gamma fp32 broadcast, data bf16).
No PE, no PSUM, no carry chain. Input ring = SP (sync), output ring = Act
(scalar); mixing directions on one ring halves its throughput.
"""
import sys

sys.path.insert(0, "/opt/trn_rl_repo")
import numpy as np

B, H, S, D = 8, 16, 4096, 128

_CACHE = {}


def _build(repeat=1, mode="full"):
    import contextlib

    import concourse.bacc as bacc
    import concourse.tile as tile
    from concourse import mybir

    f32 = mybir.dt.float32
    bf16 = mybir.dt.bfloat16

    nc = bacc.Bacc("TRN2", target_bir_lowering=False, debug=False)

    x_in = nc.declare_dram_parameter("x", [H, D, S], bf16, isOutput=False)
    g_in = nc.declare_dram_parameter("g", [D, H], f32, isOutput=False)
    y_out = nc.declare_dram_parameter("y", [H, D, S], bf16, isOutput=True)

    with tile.TileContext(nc) as tc:
        with (
            tc.tile_pool(name="const", bufs=1) as const_pool,
            tc.tile_pool(name="xp", bufs=4) as x_pool,
            tc.tile_pool(name="op", bufs=3) as out_pool,
        ):
            gt = const_pool.tile([D, H], f32)
            nc.sync.dma_start(out=gt[:], in_=g_in[:])

            xt = [None] * H
            yt = [None] * H

            def stage_in(h):
                xt[h] = x_pool.tile([D, S], bf16, name=f"xt{h}", tag="xt")
                nc.sync.dma_start(out=xt[h][:], in_=x_in[h])

            def stage_scan(h):
                yt[h] = out_pool.tile([D, S], bf16, name=f"yt{h}", tag="yt")
                if mode != "dmaonly":
                    nc.vector.tensor_tensor_scan(
                        out=yt[h][:],
                        data0=gt[:, h : h + 1].broadcast_to([D, S]),
                        data1=xt[h][:],
                        initial=0.0,
                        op0=mybir.AluOpType.mult,
                        op1=mybir.AluOpType.add,
                    )
                    src = yt[h]
                else:
                    src = xt[h]
                if mode != "scanonly":
                    nc.scalar.dma_start(out=y_out[h], in_=src[:])

            loop = (
                tc.For_i(0, repeat, 1, staggered_reset=True)
                if repeat > 1
                else contextlib.nullcontext()
            )
            with loop:
                for i in range(H + 1):
                    if i < H:
                        stage_in(i)
                    if 0 <= i - 1 < H:
                        stage_scan(i - 1)

    nc.compile()
    return nc


def _prep_inputs(tensor, gamma):
    """Full f32 (B,H,S,D) -> per-core input maps in device layout (bf16)."""
    from ml_dtypes import bfloat16

    g = np.ascontiguousarray(
        np.broadcast_to(np.asarray(gamma, np.float32)[None, :], (D, H))
    )
    xb = np.asarray(tensor).astype(bfloat16)
    in_maps = [
        {
            # (H, S, D) -> (H, D, S), contiguous per head
            "x": np.ascontiguousarray(xb[c].transpose(0, 2, 1)),
            "g": g,
        }
        for c in range(B)
    ]
    return in_maps


def _postprocess(y_cores):
    """Per-core device outputs [H, D, S] bf16 -> full (B,H,S,D) f32."""
    y = np.stack(y_cores, axis=0)  # (B, H, D, S) bf16
    return np.ascontiguousarray(y.transpose(0, 1, 3, 2)).astype(np.float32)


def _fast_callable(nc):
    """Cached jitted shard_map callable (avoids per-call retrace)."""
    import jax
    from jax.experimental.shard_map import shard_map
    from jax.sharding import Mesh, NamedSharding, PartitionSpec
    from concourse import bass2jax, mybir

    bass2jax.install_neuronx_cc_hook()
    partition_name = nc.partition_id_tensor.name if nc.partition_id_tensor else None
    in_names, out_names, out_avals, zero_outs = [], [], [], []
    for alloc in nc.m.functions[0].allocations:
        if not isinstance(alloc, mybir.MemoryLocationSet):
            continue
        name = alloc.memorylocations[0].name
        if alloc.kind == "ExternalInput":
            if name != partition_name:
                in_names.append(name)
        elif alloc.kind == "ExternalOutput":
            shape = tuple(alloc.tensor_shape)
            dtype = mybir.dt.np(alloc.dtype)
            out_avals.append(jax.core.ShapedArray(shape, dtype))
            out_names.append(name)
            zero_outs.append(np.zeros(shape, dtype))
    n_params = len(in_names)
    all_in = list(in_names) + list(out_names)
    if partition_name is not None:
        all_in.append(partition_name)

    def _body(*args):
        operands = list(args)
        if partition_name is not None:
            operands.append(bass2jax.partition_id_tensor())
        return tuple(
            bass2jax._bass_exec_p.bind(
                *operands,
                out_avals=tuple(out_avals),
                in_names=tuple(all_in),
                out_names=tuple(out_names),
                lowering_input_output_aliases=(),
                sim_require_finite=True,
                sim_require_nnan=True,
                nc=nc,
            )
        )

    devices = jax.devices()[:B]
    mesh = Mesh(np.asarray(devices), ("core",))
    specs = (PartitionSpec("core"),)
    f = jax.jit(
        shard_map(
            _body,
            mesh=mesh,
            in_specs=specs * (n_params + len(out_names)),
            out_specs=specs * len(out_names),
            check_rep=False,
        ),
        keep_unused=True,
    )
    sharding = NamedSharding(mesh, PartitionSpec("core"))
    dev_zero = [
        jax.device_put(np.zeros((B * z.shape[0], *z.shape[1:]), z.dtype), sharding)
        for z in zero_outs
    ]
    return f, in_names, out_names, out_avals, sharding, dev_zero


def _run_fast(nc, in_maps):
    import jax

    if "fast" not in _CACHE:
        _CACHE["fast"] = _fast_callable(nc)
    f, in_names, out_names, out_avals, sharding, dev_zero = _CACHE["fast"]
    concat_in = [
        jax.device_put(
            np.concatenate([np.asarray(m[nm]) for m in in_maps], axis=0), sharding
        )
        for nm in in_names
    ]
    outs = f(*concat_in, *dev_zero)
    return [
        {
            nm: np.asarray(outs[i]).reshape(B, *out_avals[i].shape)[c]
            for i, nm in enumerate(out_names)
        }
        for c in range(B)
    ]


def _run(tensor, gamma, trace=False, repeat=1):
    from concourse.bass_utils import run_bass_kernel_spmd

    key = f"nc{repeat}"
    if key not in _CACHE:
        _CACHE[key] = _build(repeat)
    nc = _CACHE[key]

    in_maps = _prep_inputs(tensor, gamma)
    if repeat == 1 and not trace:
        try:
            results = _run_fast(nc, in_maps)
            y = _postprocess([results[c]["y"] for c in range(B)])
            return y, None
        except Exception:
            pass  # fall back to the reference path below
    res = run_bass_kernel_spmd(nc, in_maps, core_ids=list(range(B)), trace=trace)
    y = _postprocess([res.results[c]["y"] for c in range(B)])
    return y, res


def kernel(tensor, gamma):
    try:
        y, _ = _run(tensor, gamma)
    except Exception:
        # transient device/pool errors: clear cached state and retry once
        _CACHE.clear()
        y, _ = _run(tensor, gamma)
    return y


# revision 33
# speedup vs baseline: 1.1348x; 1.1348x over previous
"""Discounted cumsum (B,H,S,D)=(8,16,4096,128), gamma per head, scan along S.

Hybrid: batch-parallel across 8 NeuronCores (1 batch each, all 16 heads).
bf16 I/O, host pre/post-transposes, contiguous 8KB-partition-line DMAs,
in-ring = SP, out-ring = Act.

Two per-head paths, balancing DVE vs Act/PE:
 - scan-heads (11): X^T [D=128 part, S free]; whole recurrence is one DVE
   tensor_tensor_scan (state fp32, gamma fp32 broadcast).
 - pe-heads (5): X [T=128 part = pos-in-block, (block, d) free]; two PE
   passes (block sums via w^T X, carry matmul folded into row 0, then
   A @ X'), PSUM drains + staging copies on Act, reshapes on gpsimd.
"""
import sys

sys.path.insert(0, "/opt/trn_rl_repo")
import numpy as np

B, H, S, D = 8, 16, 4096, 128
T = 128
KB = S // T
TILE = 4 * T
NT = S // TILE

PE_HEADS = (1, 4, 7, 10, 13)

_CACHE = {}


def _build(repeat=1, mode="full"):
    import contextlib

    import concourse.bacc as bacc
    import concourse.tile as tile
    from concourse import mybir

    f32 = mybir.dt.float32
    bf16 = mybir.dt.bfloat16

    nc = bacc.Bacc("TRN2", target_bir_lowering=False, debug=False)

    x_in = nc.declare_dram_parameter("x", [H, T, S], bf16, isOutput=False)
    g_in = nc.declare_dram_parameter("g", [D, H], f32, isOutput=False)
    at_in = nc.declare_dram_parameter("at", [T, H * T], bf16, isOutput=False)
    w_in = nc.declare_dram_parameter("w", [T, H], bf16, isOutput=False)
    aug_in = nc.declare_dram_parameter("aug", [KB, H * KB], bf16, isOutput=False)
    y_out = nc.declare_dram_parameter("y", [H, T, S], bf16, isOutput=True)

    with tile.TileContext(nc) as tc:
        with (
            tc.tile_pool(name="const", bufs=1) as const_pool,
            tc.tile_pool(name="xp", bufs=6) as x_pool,
            tc.tile_pool(name="op", bufs=4) as out_pool,
            tc.tile_pool(name="sflp", bufs=2) as sfl_pool,
            tc.tile_pool(name="sxp", bufs=2) as sx_pool,
            tc.tile_pool(name="rp", bufs=2) as r_pool,
            tc.tile_pool(name="sps", bufs=2, space="PSUM") as s_psum,
            tc.tile_pool(name="rps", bufs=1, space="PSUM") as r_psum,
            tc.tile_pool(name="yps", bufs=4, space="PSUM") as y_psum,
        ):
            gt = const_pool.tile([D, H], f32)
            at_sb = const_pool.tile([T, H * T], bf16)
            w_sb = const_pool.tile([T, H], bf16)
            aug_sb = const_pool.tile([KB, H * KB], bf16)
            nc.sync.dma_start(out=gt[:], in_=g_in[:])
            nc.sync.dma_start(out=at_sb[:], in_=at_in[:])
            nc.sync.dma_start(out=w_sb[:], in_=w_in[:])
            nc.sync.dma_start(out=aug_sb[:], in_=aug_in[:])

            xt = [None] * H
            yt = [None] * H
            s32 = [None] * H

            def stage_in(h):
                xt[h] = x_pool.tile([T, S], bf16, name=f"xt{h}", tag="xt")
                nc.sync.dma_start(out=xt[h][:], in_=x_in[h])

            def stage_scan(h):
                yt[h] = out_pool.tile([D, S], bf16, name=f"yt{h}", tag="yt")
                nc.vector.tensor_tensor_scan(
                    out=yt[h][:],
                    data0=gt[:, h : h + 1].broadcast_to([D, S]),
                    data1=xt[h][:],
                    initial=0.0,
                    op0=mybir.AluOpType.mult,
                    op1=mybir.AluOpType.add,
                )
                nc.scalar.dma_start(out=y_out[h], in_=yt[h][:])

            def stage_s(h):
                s_fl = sfl_pool.tile([1, S], bf16, name="sfl", tag="sfl")
                for t in range(NT):
                    s_ps = s_psum.tile([1, TILE], f32, name="sps", tag="sps")
                    nc.tensor.matmul(
                        s_ps[:],
                        w_sb[:, h : h + 1],
                        xt[h][:, t * TILE : (t + 1) * TILE],
                        start=True,
                        stop=True,
                    )
                    nc.scalar.copy(
                        out=s_fl[0:1, t * TILE : (t + 1) * TILE], in_=s_ps[:]
                    )
                s32[h] = sx_pool.tile([KB, D], bf16, name=f"s32{h}", tag="s32")
                # shape-mismatched APs (no rearrange): rearranged SWDGE
                # descriptors scatter wrong on HW (sim models them fine)
                nc.gpsimd.dma_start(out=s32[h][:], in_=s_fl[:])

            def stage_r(h):
                # r = g*AB @ s : carry scaled into row-0 units
                r_ps = r_psum.tile([KB, D], f32, name="rps", tag="rps")
                nc.tensor.matmul(
                    r_ps[:],
                    aug_sb[:, h * KB : (h + 1) * KB],
                    s32[h][:],
                    start=True,
                    stop=True,
                )
                r32 = r_pool.tile([KB, D], bf16, name="r32", tag="r32")
                nc.scalar.copy(out=r32[:], in_=r_ps[:])
                # accumulate r into row 0 of xt (x'_0 = x_0 + g*c)
                nc.gpsimd.dma_start(
                    out=xt[h][0:1, :],
                    in_=r32[:],
                    accum_op=mybir.AluOpType.add,
                )

            def stage_y(h):
                yt[h] = out_pool.tile([T, S], bf16, name=f"yt{h}", tag="yt")
                for t in range(NT):
                    y_ps = y_psum.tile([T, TILE], f32, name="yps", tag="yps")
                    nc.tensor.matmul(
                        y_ps[:],
                        at_sb[:, h * T : (h + 1) * T],
                        xt[h][:, t * TILE : (t + 1) * TILE],
                        start=True,
                        stop=True,
                    )
                    nc.scalar.copy(
                        out=yt[h][:, t * TILE : (t + 1) * TILE], in_=y_ps[:]
                    )
                nc.scalar.dma_start(out=y_out[h], in_=yt[h][:])

            loop = (
                tc.For_i(0, repeat, 1, staggered_reset=True)
                if repeat > 1
                else contextlib.nullcontext()
            )
            with loop:
                if mode == "dmaonly":
                    for i in range(H):
                        stage_in(i)
                        nc.scalar.dma_start(out=y_out[i], in_=xt[i][:])
                else:
                    for i in range(H + 4):
                        if i < H:
                            stage_in(i)
                        if 0 <= i - 1 < H:
                            h = i - 1
                            if h in PE_HEADS:
                                stage_s(h)
                            else:
                                stage_scan(h)
                        if 0 <= i - 2 < H and (i - 2) in PE_HEADS:
                            stage_r(i - 2)
                        if 0 <= i - 4 < H and (i - 4) in PE_HEADS:
                            stage_y(i - 4)

    nc.compile()
    return nc


def _constants(gamma):
    from ml_dtypes import bfloat16

    g = gamma.astype(np.float64)  # [H]
    i = np.arange(T)
    diff = i[:, None] - i[None, :]  # [i, s]
    at = np.zeros((T, H * T), np.float64)
    w = np.zeros((T, H), np.float64)
    aug = np.zeros((KB, H * KB), np.float64)
    k = np.arange(KB)
    kdiff = k[None, :] - k[:, None] - 1  # [j, k] -> k-1-j
    for h in range(H):
        gh = g[h]
        a_h = np.where(diff >= 0, gh ** np.maximum(diff, 0), 0.0)  # [i, s]
        at[:, h * T : (h + 1) * T] = a_h.T
        w[:, h] = gh ** (T - 1 - i)
        G = gh ** T
        aug[:, h * KB : (h + 1) * KB] = gh * np.where(
            kdiff >= 0, G ** np.maximum(kdiff, 0), 0.0
        )
    return at.astype(bfloat16), w.astype(bfloat16), aug.astype(bfloat16)


def _prep_inputs(tensor, gamma):
    """Full f32 (B,H,S,D) -> per-core input maps in device layout (bf16)."""
    from ml_dtypes import bfloat16

    at, w, aug = _constants(np.asarray(gamma))
    g = np.ascontiguousarray(
        np.broadcast_to(np.asarray(gamma, np.float32)[None, :], (D, H))
    )
    xb = np.asarray(tensor).astype(bfloat16)
    in_maps = []
    for c in range(B):
        xc = np.empty((H, T, S), bfloat16)
        for h in range(H):
            if h in PE_HEADS:
                # (S, D) -> (KB, T, D) -> (T, KB, D) -> (T, S)
                xc[h] = (
                    xb[c, h]
                    .reshape(KB, T, D)
                    .transpose(1, 0, 2)
                    .reshape(T, S)
                )
            else:
                xc[h] = xb[c, h].T  # (S, D) -> (D, S)
        in_maps.append({"x": xc, "g": g, "at": at, "w": w, "aug": aug})
    return in_maps


def _postprocess(y_cores):
    """Per-core device outputs [H, T, S] bf16 -> full (B,H,S,D) f32."""
    y = np.stack(y_cores, axis=0)  # (B, H, 128, 4096) bf16
    out = np.empty((B, H, S, D), np.float32)
    for h in range(H):
        if h in PE_HEADS:
            out[:, h] = (
                y[:, h]
                .reshape(B, T, KB, D)
                .transpose(0, 2, 1, 3)
                .reshape(B, S, D)
                .astype(np.float32)
            )
        else:
            out[:, h] = y[:, h].transpose(0, 2, 1).astype(np.float32)
    return out


def _fast_callable(nc):
    """Cached jitted shard_map callable (avoids per-call retrace)."""
    import jax
    from jax.experimental.shard_map import shard_map
    from jax.sharding import Mesh, NamedSharding, PartitionSpec
    from concourse import bass2jax, mybir

    bass2jax.install_neuronx_cc_hook()
    partition_name = nc.partition_id_tensor.name if nc.partition_id_tensor else None
    in_names, out_names, out_avals, zero_outs = [], [], [], []
    for alloc in nc.m.functions[0].allocations:
        if not isinstance(alloc, mybir.MemoryLocationSet):
            continue
        name = alloc.memorylocations[0].name
        if alloc.kind == "ExternalInput":
            if name != partition_name:
                in_names.append(name)
        elif alloc.kind == "ExternalOutput":
            shape = tuple(alloc.tensor_shape)
            dtype = mybir.dt.np(alloc.dtype)
            out_avals.append(jax.core.ShapedArray(shape, dtype))
            out_names.append(name)
            zero_outs.append(np.zeros(shape, dtype))
    n_params = len(in_names)
    all_in = list(in_names) + list(out_names)
    if partition_name is not None:
        all_in.append(partition_name)

    def _body(*args):
        operands = list(args)
        if partition_name is not None:
            operands.append(bass2jax.partition_id_tensor())
        return tuple(
            bass2jax._bass_exec_p.bind(
                *operands,
                out_avals=tuple(out_avals),
                in_names=tuple(all_in),
                out_names=tuple(out_names),
                lowering_input_output_aliases=(),
                sim_require_finite=True,
                sim_require_nnan=True,
                nc=nc,
            )
        )

    devices = jax.devices()[:B]
    mesh = Mesh(np.asarray(devices), ("core",))
    specs = (PartitionSpec("core"),)
    f = jax.jit(
        shard_map(
            _body,
            mesh=mesh,
            in_specs=specs * (n_params + len(out_names)),
            out_specs=specs * len(out_names),
            check_rep=False,
        ),
        keep_unused=True,
    )
    sharding = NamedSharding(mesh, PartitionSpec("core"))
    dev_zero = [
        jax.device_put(np.zeros((B * z.shape[0], *z.shape[1:]), z.dtype), sharding)
        for z in zero_outs
    ]
    return f, in_names, out_names, out_avals, sharding, dev_zero


def _run_fast(nc, in_maps):
    import jax

    if "fast" not in _CACHE:
        _CACHE["fast"] = _fast_callable(nc)
    f, in_names, out_names, out_avals, sharding, dev_zero = _CACHE["fast"]
    concat_in = [
        jax.device_put(
            np.concatenate([np.asarray(m[nm]) for m in in_maps], axis=0), sharding
        )
        for nm in in_names
    ]
    outs = f(*concat_in, *dev_zero)
    return [
        {
            nm: np.asarray(outs[i]).reshape(B, *out_avals[i].shape)[c]
            for i, nm in enumerate(out_names)
        }
        for c in range(B)
    ]


def _run(tensor, gamma, trace=False, repeat=1):
    from concourse.bass_utils import run_bass_kernel_spmd

    key = f"nc{repeat}"
    if key not in _CACHE:
        _CACHE[key] = _build(repeat)
    nc = _CACHE[key]

    in_maps = _prep_inputs(tensor, gamma)
    if repeat == 1 and not trace:
        try:
            results = _run_fast(nc, in_maps)
            y = _postprocess([results[c]["y"] for c in range(B)])
            return y, None
        except Exception:
            pass  # fall back to the reference path below
    res = run_bass_kernel_spmd(nc, in_maps, core_ids=list(range(B)), trace=trace)
    y = _postprocess([res.results[c]["y"] for c in range(B)])
    return y, res


def kernel(tensor, gamma):
    try:
        y, _ = _run(tensor, gamma)
    except Exception:
        # transient device/pool errors: clear cached state and retry once
        _CACHE.clear()
        y, _ = _run(tensor, gamma)
    return y


# revision 36
# speedup vs baseline: 1.5741x; 1.3870x over previous
"""Discounted cumsum (B,H,S,D)=(8,16,4096,128), gamma per head, scan along S.

Strategy: batch-parallel across 8 NeuronCores (1 batch each, all 16 heads).
HBM traffic is the roofline, so I/O is bf16 (rel-err budget 2e-2; bf16 I/O
lands ~3e-3) and the host pre/post-transposes so every device DMA moves
fully contiguous 8KB partition lines.

Per head, X is laid out transposed [D=128 partitions, S=4096 free] and the
whole recurrence y[:, t] = g*y[:, t-1] + x[:, t] runs as ONE DVE
tensor_tensor_scan instruction (state fp32, gamma fp32 broadcast, data bf16).
No PE, no PSUM, no carry chain. Input ring = SP (sync), output ring = Act
(scalar); mixing directions on one ring halves its throughput.
"""
import sys

sys.path.insert(0, "/opt/trn_rl_repo")
import numpy as np

B, H, S, D = 8, 16, 4096, 128

_CACHE = {}


def _build(repeat=1, mode="full"):
    import contextlib

    import concourse.bacc as bacc
    import concourse.tile as tile
    from concourse import mybir

    f32 = mybir.dt.float32
    bf16 = mybir.dt.bfloat16

    nc = bacc.Bacc("TRN2", target_bir_lowering=False, debug=False)

    x_in = nc.declare_dram_parameter("x", [H, D, S], bf16, isOutput=False)
    g_in = nc.declare_dram_parameter("g", [D, H], f32, isOutput=False)
    y_out = nc.declare_dram_parameter("y", [H, D, S], bf16, isOutput=True)

    with tile.TileContext(nc) as tc:
        with (
            tc.tile_pool(name="const", bufs=1) as const_pool,
            tc.tile_pool(name="xp", bufs=6) as x_pool,
            tc.tile_pool(name="op", bufs=4) as out_pool,
        ):
            gt = const_pool.tile([D, H], f32)
            nc.sync.dma_start(out=gt[:], in_=g_in[:])

            xt = [None] * H
            yt = [None] * H

            def stage_in(h):
                xt[h] = x_pool.tile([D, S], bf16, name=f"xt{h}", tag="xt")
                nc.sync.dma_start(out=xt[h][:], in_=x_in[h])

            def stage_scan(h):
                yt[h] = out_pool.tile([D, S], bf16, name=f"yt{h}", tag="yt")
                if mode != "dmaonly":
                    nc.vector.tensor_tensor_scan(
                        out=yt[h][:],
                        data0=gt[:, h : h + 1].broadcast_to([D, S]),
                        data1=xt[h][:],
                        initial=0.0,
                        op0=mybir.AluOpType.mult,
                        op1=mybir.AluOpType.add,
                    )
                    src = yt[h]
                else:
                    src = xt[h]
                if mode not in ("scanonly", "scanpure"):
                    nc.scalar.dma_start(out=y_out[h], in_=src[:])

            loop = (
                tc.For_i(0, repeat, 1, staggered_reset=True)
                if repeat > 1
                else contextlib.nullcontext()
            )
            with loop:
                if mode == "scanpure":
                    xconst = const_pool.tile([D, S], bf16)
                    nc.vector.memset(xconst[:], 0.125)
                    for h in range(H):
                        xt[h] = xconst
                        stage_scan(h)
                elif mode == "full2x":
                    for rep in range(2):
                        for i in range(H + 1):
                            if i < H:
                                stage_in(i)
                            if 0 <= i - 1 < H:
                                stage_scan(i - 1)
                else:
                    for i in range(H + 1):
                        if i < H:
                            stage_in(i)
                        if 0 <= i - 1 < H:
                            stage_scan(i - 1)

    nc.compile()
    return nc


def _prep_inputs(tensor, gamma):
    """Full f32 (B,H,S,D) -> per-core input maps in device layout (bf16)."""
    from ml_dtypes import bfloat16

    g = np.ascontiguousarray(
        np.broadcast_to(np.asarray(gamma, np.float32)[None, :], (D, H))
    )
    xb = np.asarray(tensor).astype(bfloat16)
    in_maps = [
        {
            # (H, S, D) -> (H, D, S), contiguous per head
            "x": np.ascontiguousarray(xb[c].transpose(0, 2, 1)),
            "g": g,
        }
        for c in range(B)
    ]
    return in_maps


def _postprocess(y_cores):
    """Per-core device outputs [H, D, S] bf16 -> full (B,H,S,D) f32."""
    y = np.stack(y_cores, axis=0)  # (B, H, D, S) bf16
    return np.ascontiguousarray(y.transpose(0, 1, 3, 2)).astype(np.float32)


def _fast_callable(nc):
    """Cached jitted shard_map callable (avoids per-call retrace)."""
    import jax
    from jax.experimental.shard_map import shard_map
    from jax.sharding import Mesh, NamedSharding, PartitionSpec
    from concourse import bass2jax, mybir

    bass2jax.install_neuronx_cc_hook()
    partition_name = nc.partition_id_tensor.name if nc.partition_id_tensor else None
    in_names, out_names, out_avals, zero_outs = [], [], [], []
    for alloc in nc.m.functions[0].allocations:
        if not isinstance(alloc, mybir.MemoryLocationSet):
            continue
        name = alloc.memorylocations[0].name
        if alloc.kind == "ExternalInput":
            if name != partition_name:
                in_names.append(name)
        elif alloc.kind == "ExternalOutput":
            shape = tuple(alloc.tensor_shape)
            dtype = mybir.dt.np(alloc.dtype)
            out_avals.append(jax.core.ShapedArray(shape, dtype))
            out_names.append(name)
            zero_outs.append(np.zeros(shape, dtype))
    n_params = len(in_names)
    all_in = list(in_names) + list(out_names)
    if partition_name is not None:
        all_in.append(partition_name)

    def _body(*args):
        operands = list(args)
        if partition_name is not None:
            operands.append(bass2jax.partition_id_tensor())
        return tuple(
            bass2jax._bass_exec_p.bind(
                *operands,
                out_avals=tuple(out_avals),
                in_names=tuple(all_in),
                out_names=tuple(out_names),
                lowering_input_output_aliases=(),
                sim_require_finite=True,
                sim_require_nnan=True,
                nc=nc,
            )
        )

    devices = jax.devices()[:B]
    mesh = Mesh(np.asarray(devices), ("core",))
    specs = (PartitionSpec("core"),)
    f = jax.jit(
        shard_map(
            _body,
            mesh=mesh,
            in_specs=specs * (n_params + len(out_names)),
            out_specs=specs * len(out_names),
            check_rep=False,
        ),
        keep_unused=True,
    )
    sharding = NamedSharding(mesh, PartitionSpec("core"))
    dev_zero = [
        jax.device_put(np.zeros((B * z.shape[0], *z.shape[1:]), z.dtype), sharding)
        for z in zero_outs
    ]
    return f, in_names, out_names, out_avals, sharding, dev_zero


def _run_fast(nc, in_maps):
    import jax

    if "fast" not in _CACHE:
        _CACHE["fast"] = _fast_callable(nc)
    f, in_names, out_names, out_avals, sharding, dev_zero = _CACHE["fast"]
    concat_in = [
        jax.device_put(
            np.concatenate([np.asarray(m[nm]) for m in in_maps], axis=0), sharding
        )
        for nm in in_names
    ]
    outs = f(*concat_in, *dev_zero)
    return [
        {
            nm: np.asarray(outs[i]).reshape(B, *out_avals[i].shape)[c]
            for i, nm in enumerate(out_names)
        }
        for c in range(B)
    ]


def _run(tensor, gamma, trace=False, repeat=1):
    from concourse.bass_utils import run_bass_kernel_spmd

    key = f"nc{repeat}"
    if key not in _CACHE:
        _CACHE[key] = _build(repeat)
    nc = _CACHE[key]

    in_maps = _prep_inputs(tensor, gamma)
    if repeat == 1 and not trace:
        try:
            results = _run_fast(nc, in_maps)
            y = _postprocess([results[c]["y"] for c in range(B)])
            return y, None
        except Exception:
            pass  # fall back to the reference path below
    res = run_bass_kernel_spmd(nc, in_maps, core_ids=list(range(B)), trace=trace)
    y = _postprocess([res.results[c]["y"] for c in range(B)])
    return y, res


def kernel(tensor, gamma):
    try:
        y, _ = _run(tensor, gamma)
    except Exception:
        # transient device/pool errors: clear cached state and retry once
        _CACHE.clear()
        y, _ = _run(tensor, gamma)
    return y
